# revision 9
# baseline (speedup 1.0000x reference)
"""Trainium2 Bass kernel for nn_CombineLoss (focal + dice + edge loss).

Sharding: data-parallel over the batch dim B=8 -> one batch image per
NeuronCore.  Each core computes 9 partial sums per 128-row tile (8 tiles);
the host combines them in float64.

Math notes (per head; heads = 2 softmaxed prediction heads + 1 raw-prob head):
  - softmax over C=2 channels:  p1 = sigmoid(x1-x0), p0 = sigmoid(x0-x1),
    so log p_t = log(sigmoid((2t-1)*(x1-x0))).
  - log fusion: focal and edge only need sum_h log p_t^(h) (sigma weights
    shared across heads), so ln(pt0*pt1*ptd) replaces three Ln ops.
  - 7x7 "circular" box filter (29 taps): decomposed by row offset dy:
      29*ave(p,x) = [H5(p,x) + t(p,x-3) + t(p,x+3)]   (dy=0 row, 7 taps)
                  + H5(p+/-1,x) + H5(p+/-2,x)         (|dy|=1,2 rows)
                  + t(p+/-3,x)                        (|dy|=3 rows)
    where H5 = 5-wide horizontal window sum (computed with 4 shifted bf16
    adds, exact small integers).  The vertical (partition) combination runs
    on the TensorEngine as banded matmuls; the -29*t center term is folded
    into the A33 center band so PSUM ends up holding 29*(ave-t) and
    |PSUM| = 29*at.
  - scalar-engine work is chunked (all sigmoids of a 4-tile chunk, then all
    Lns) because no activation table holds both Sigmoid and Ln.
  - per-tile sums ride on activation / tensor_scalar / scalar_tensor_tensor
    accum_out; final reduction happens on host in float64.
"""

import numpy as np

# ---------------------------------------------------------------------------
# problem constants (hardcoded per contest rules)
N_HEADS_PRED = 2   # predictions list dim
B = 8              # batch -> one per core
C = 2
H = 1024
W = 1024
P = 128            # partitions
NT = H // P        # 8 row tiles per image
TBW = 1032         # padded width of the bf16 target slots (3 left, 5 right)
SLOTS = 9          # stat slots per tile
NSLOT = NT * SLOTS
EPS = 1e-10
N_CORES = 8
CHUNK = 4          # tiles per scalar-table chunk (sigmoids ... lns)

# stat slot layout (per tile, base = 9*i):
#  +0  sum(t)
#  +1  pred0 sum(pt)     +2  pred1 sum(pt)
#  +3  sum(lp_all)       (ln(pt0*pt1*ptd), all 3 heads)
#  +4  pred0 sum(pt*t)   +5  pred1 sum(pt*t)
#  +6  sum(lp_all*at29)
#  +7  diss  sum(p1)     +8  diss  sum(p1*t)

_CACHE = {}


def _ensure_path():
    try:
        import concourse  # noqa: F401
    except ImportError:
        import sys
        for p in ("/opt/trn_rl_repo", "/root/.axon_site/_ro/trn_rl_repo"):
            if p not in sys.path:
                sys.path.insert(0, p)


# band indices
A25C, A25P, A25N, A33C, A33P, A33N, I128 = range(7)


def _make_bands():
    """7 [128,128] banded matrices, bf16 (all entries small ints -> exact).

    Built as stat[m, p]: contraction index m (source row), output partition p.
    matmul computes out[p, j] = sum_m stat[m, p] * moving[m, j].
    """
    import ml_dtypes
    idx = np.arange(P)
    d = idx[:, None] - idx[None, :]  # d[m, p] = m - p
    a25c = (np.abs(d) <= 2).astype(np.float32)
    a25p = ((d >= 126) & (d <= 127)).astype(np.float32)
    a25n = ((d <= -126) & (d >= -127)).astype(np.float32)
    a33c = (np.abs(d) == 3).astype(np.float32) - 29.0 * (d == 0)
    a33p = (d == 125).astype(np.float32)
    a33n = (d == -125).astype(np.float32)
    i128 = (d == 0).astype(np.float32)
    bands = np.stack([a25c, a25p, a25n, a33c, a33p, a33n, i128])
    return bands.astype(ml_dtypes.bfloat16)


def _build_nc():
    _ensure_path()
    import concourse.mybir as mybir
    from concourse import bacc
    from concourse.tile import TileContext

    f32 = mybir.dt.float32
    bf16 = mybir.dt.bfloat16
    i32 = mybir.dt.int32
    Alu = mybir.AluOpType
    Act = mybir.ActivationFunctionType

    nc = bacc.Bacc()
    preds = nc.dram_tensor("preds", [N_HEADS_PRED, C, H, W], f32,
                           kind="ExternalInput")
    diss = nc.dram_tensor("diss", [C, H, W], f32, kind="ExternalInput")
    target = nc.dram_tensor("target", [H, W], i32, kind="ExternalInput")
    bands = nc.dram_tensor("bands", [7, P, P], bf16, kind="ExternalInput")
    stats_out = nc.dram_tensor("stats", [P, NSLOT], f32, kind="ExternalOutput")

    with TileContext(nc) as tc:
        with (
            tc.tile_pool(name="const", bufs=1) as constp,
            tc.tile_pool(name="res", bufs=1) as resp,
            tc.tile_pool(name="io", bufs=9) as iop,
            tc.tile_pool(name="t32pool", bufs=3) as t32p,
            tc.tile_pool(name="mid", bufs=6) as midp,
            tc.tile_pool(name="keep", bufs=5) as keepp,
            tc.tile_pool(name="ps", bufs=2, space="PSUM") as psump,
        ):
            stats_sb = constp.tile([P, NSLOT], f32)
            eps_t = constp.tile([P, 1], f32)
            nc.gpsimd.memset(eps_t[:], EPS)
            tb_all = resp.tile([P, NT, TBW], bf16)
            h5_all = resp.tile([P, NT, W], bf16)
            nc.gpsimd.memset(tb_all[:], 0.0)

            # per-tile state captured by phaseA, consumed by later stages
            state = {}

            def phase_a(i):
                """DMA loads + target-derived planes (tb, sgn, H5)."""
                t32 = t32p.tile([P, W], i32, tag="t32", name=f"t32_{i}")
                nc.sync.dma_start(out=t32, in_=target[i * P:(i + 1) * P, :])
                xt0 = iop.tile([P, C, W], f32, tag="xin", name=f"xt0_{i}")
                nc.sync.dma_start(
                    out=xt0, in_=preds[0, :, i * P:(i + 1) * P, :].rearrange(
                        "c h w -> h c w"))
                xt1 = iop.tile([P, C, W], f32, tag="xin", name=f"xt1_{i}")
                nc.sync.dma_start(
                    out=xt1, in_=preds[1, :, i * P:(i + 1) * P, :].rearrange(
                        "c h w -> h c w"))
                dt = iop.tile([P, C, W], f32, tag="xin", name=f"dt_{i}")
                nc.sync.dma_start(
                    out=dt, in_=diss[:, i * P:(i + 1) * P, :].rearrange(
                        "c h w -> h c w"))

                base = i * SLOTS
                tb_i = tb_all[:, i, 3:3 + W]
                # i32 -> bf16 {0,1} plane, accumulate sum(t) (Copy: no table)
                nc.scalar.activation(
                    tb_i, t32, Act.Copy,
                    accum_out=stats_sb[:, base + 0:base + 1])
                sgn = midp.tile([P, W], bf16, tag="sgn", bufs=3,
                                name=f"sgn_{i}")
                nc.gpsimd.tensor_scalar(sgn, t32, 2.0, -1.0,
                                        Alu.mult, Alu.add)
                # H5 = 5-wide horizontal sum of t, exact ints in bf16.
                # 3 ops via pair-reuse: P2(x)=t(x)+t(x+1) on a padded range,
                # P4(x)=P2(x-2)+P2(x), H5=P4+t(x+2).
                tp = tb_all[:, i, :]
                p2 = midp.tile([P, W + 4], bf16, tag="p2", bufs=2,
                               name=f"p2_{i}")  # col j <-> x = j-2
                nc.vector.scalar_tensor_tensor(
                    p2, tp[:, 1:1 + W + 4], 0.0, tp[:, 2:2 + W + 4],
                    Alu.bypass, Alu.add)
                p4 = midp.tile([P, W], bf16, tag="scr", name=f"p4_{i}")
                nc.vector.scalar_tensor_tensor(
                    p4, p2[:, 0:W], 0.0, p2[:, 2:2 + W],
                    Alu.bypass, Alu.add)
                nc.vector.scalar_tensor_tensor(
                    h5_all[:, i, :], p4, 0.0, tp[:, 5:5 + W],
                    Alu.bypass, Alu.add)
                state[i] = (t32, xt0, xt1, dt, sgn)

            def conv_elem(i):
                """Conv on PE + elementwise pipeline up to prod/at29."""
                base = i * SLOTS
                t32, xt0, xt1, dt, sgn = state.pop(i)
                tb_i = tb_all[:, i, 3:3 + W]

                # PE: psum = 29*(ave - t), exact integers
                vps = psump.tile([P, W], f32, tag="vps", name=f"vps_{i}")
                for h in range(2):
                    w0 = h * 512
                    mms = [(A25C, h5_all[:, i, w0:w0 + 512])]
                    if i > 0:
                        mms.append((A25P, h5_all[:, i - 1, w0:w0 + 512]))
                    if i < NT - 1:
                        mms.append((A25N, h5_all[:, i + 1, w0:w0 + 512]))
                    mms.append((A33C, tb_all[:, i, 3 + w0:3 + w0 + 512]))
                    if i > 0:
                        mms.append((A33P, tb_all[:, i - 1, 3 + w0:3 + w0 + 512]))
                    if i < NT - 1:
                        mms.append((A33N, tb_all[:, i + 1, 3 + w0:3 + w0 + 512]))
                    # dy=0 row: t(x-3) + t(x+3) via identity band
                    mms.append((I128, tb_all[:, i, w0:w0 + 512]))
                    mms.append((I128, tb_all[:, i, 6 + w0:6 + w0 + 512]))
                    for j, (bk, mv) in enumerate(mms):
                        nc.tensor.matmul(
                            vps[:, w0:w0 + 512], bands_sb[:, bk, :], mv,
                            start=(j == 0), stop=(j == len(mms) - 1))

                at29 = keepp.tile([P, W], bf16, tag="at29", name=f"at29_{i}")
                nc.scalar.activation(at29, vps, Act.Abs)  # = 29*|t-ave|

                # s = (x1-x0)*sgn per pred head; d on Pool, s on DVE
                d0 = midp.tile([P, W], bf16, tag="scr", name=f"d0_{i}")
                nc.gpsimd.tensor_tensor(
                    d0, xt0[:, 1, :], xt0[:, 0, :], Alu.subtract)
                s0 = midp.tile([P, W], bf16, tag="s", bufs=3, name=f"s0_{i}")
                nc.vector.scalar_tensor_tensor(
                    s0, d0, 0.0, sgn, Alu.bypass, Alu.mult)
                d1 = midp.tile([P, W], bf16, tag="scr", name=f"d1_{i}")
                nc.gpsimd.tensor_tensor(
                    d1, xt1[:, 1, :], xt1[:, 0, :], Alu.subtract)
                s1 = midp.tile([P, W], bf16, tag="s", bufs=3, name=f"s1_{i}")
                nc.vector.scalar_tensor_tensor(
                    s1, d1, 0.0, sgn, Alu.bypass, Alu.mult)

                pt0 = midp.tile([P, W], bf16, tag="pt", bufs=4,
                                name=f"pt0_{i}")
                nc.scalar.activation(
                    pt0, s0, Act.Sigmoid,
                    accum_out=stats_sb[:, base + 1:base + 2])
                pt1 = midp.tile([P, W], bf16, tag="pt", bufs=4,
                                name=f"pt1_{i}")
                nc.scalar.activation(
                    pt1, s1, Act.Sigmoid,
                    accum_out=stats_sb[:, base + 2:base + 3])

                qt0 = midp.tile([P, W], bf16, tag="scr", name=f"qt0_{i}")
                nc.vector.scalar_tensor_tensor(
                    qt0, pt0, 0.0, tb_i, Alu.bypass, Alu.mult,
                    accum_out=stats_sb[:, base + 4:base + 5])
                qt1 = midp.tile([P, W], bf16, tag="scr", name=f"qt1_{i}")
                nc.vector.scalar_tensor_tensor(
                    qt1, pt1, 0.0, tb_i, Alu.bypass, Alu.mult,
                    accum_out=stats_sb[:, base + 5:base + 6])

                # diss: ptd = t ? dt1 : dt0 (exact select); copy on Scalar
                ptd = midp.tile([P, W], f32, tag="ptd", bufs=3,
                                name=f"ptd_{i}")
                nc.scalar.activation(ptd, dt[:, 0, :], Act.Copy)
                nc.vector.copy_predicated(ptd, t32, dt[:, 1, :])
                pp = midp.tile([P, W], bf16, tag="scr", name=f"pp_{i}")
                nc.gpsimd.tensor_tensor(pp, pt0, pt1, Alu.mult)
                prod = keepp.tile([P, W], bf16, tag="prod", name=f"prod_{i}")
                nc.vector.scalar_tensor_tensor(
                    prod, pp, 0.0, ptd, Alu.bypass, Alu.mult)

                # diss dice sums on DVE (accum-reduce is DVE-only)
                scr7 = midp.tile([P, W], bf16, tag="scr", name=f"scr7_{i}")
                nc.vector.tensor_scalar(
                    scr7, dt[:, 1, :], 1.0, 0.0, Alu.mult, Alu.add,
                    accum_out=stats_sb[:, base + 7:base + 8])
                scr8 = midp.tile([P, W], bf16, tag="scr", name=f"scr8_{i}")
                nc.vector.scalar_tensor_tensor(
                    scr8, dt[:, 1, :], 0.0, tb_i, Alu.bypass, Alu.mult,
                    accum_out=stats_sb[:, base + 8:base + 9])
                state[("pr", i)] = (prod, at29)

            def phase_ln(i):
                """Deferred Ln + edge product-reduce (chunked for tables)."""
                base = i * SLOTS
                prod, at29 = state.pop(("pr", i))
                lnp = midp.tile([P, W], bf16, tag="lnp", bufs=2,
                                name=f"lnp_{i}")
                nc.scalar.activation(
                    lnp, prod, Act.Ln, bias=eps_t[:, 0:1],
                    accum_out=stats_sb[:, base + 3:base + 4])
                escr = midp.tile([P, W], bf16, tag="scr", name=f"escr_{i}")
                nc.vector.scalar_tensor_tensor(
                    escr, lnp, 0.0, at29, Alu.bypass, Alu.mult,
                    accum_out=stats_sb[:, base + 6:base + 7])

            # ---- issue the pipeline ------------------------------------
            phase_a(0)
            bands_sb = constp.tile([P, 7, P], bf16)
            for k in range(7):
                nc.sync.dma_start(out=bands_sb[:, k, :], in_=bands[k])
            for i in range(NT):
                if i + 1 < NT:
                    phase_a(i + 1)
                conv_elem(i)
                if i % CHUNK == CHUNK - 1:
                    for j in range(i - CHUNK + 1, i + 1):
                        phase_ln(j)

            nc.sync.dma_start(out=stats_out[:], in_=stats_sb[:])

    nc.finalize()
    return nc


def get_program():
    if "nc" not in _CACHE:
        _CACHE["nc"] = _build_nc()
    return _CACHE["nc"]


def make_in_maps(predictions, Diss, target):
    bands = _make_bands()
    in_maps = []
    for c in range(N_CORES):
        in_maps.append({
            "preds": np.ascontiguousarray(predictions[:, c], dtype=np.float32),
            "diss": np.ascontiguousarray(Diss[0, c], dtype=np.float32),
            "target": np.ascontiguousarray(target[c], dtype=np.int32),
            "bands": bands,
        })
    return in_maps


def assemble(stats_list, sigma, diff):
    """Combine per-core [128, 72] stats into the scalar loss (float64)."""
    HW = float(H * W)
    focal_all = 0.0
    edge_all = 0.0
    dice_sum = 0.0  # sum over cores and heads of 2I/(U+eps)
    for st in stats_list:
        g = st.astype(np.float64).sum(axis=0).reshape(NT, SLOTS).sum(axis=0)
        s_t = g[0]
        n0 = HW - s_t
        focal_all += -g[3]
        edge_all += -g[6] / 29.0
        for hh in range(2):
            spt, i_sum = g[1 + hh], g[4 + hh]
            sp1 = n0 + 2.0 * i_sum - spt
            dice_sum += 2.0 * i_sum / (sp1 + s_t + EPS)
        sp1d, i_d = g[7], g[8]
        dice_sum += 2.0 * i_d / (sp1d + s_t + EPS)

    sig2 = np.asarray(sigma, np.float64) ** 2
    denom = float(N_CORES) * HW
    loss = (focal_all / denom / sig2[0]
            + (3.0 - dice_sum / float(N_CORES)) / sig2[1]
            + edge_all / denom / sig2[2])
    loss += float(diff)
    loss += float(np.sum(np.log(sig2))) / 2.0
    return np.float32(loss)


def run_on_hw(predictions, Diss, target, trace=False):
    _ensure_path()
    from concourse.bass_utils import run_bass_kernel_spmd
    nc = get_program()
    in_maps = make_in_maps(predictions, Diss, target)
    res = run_bass_kernel_spmd(nc, in_maps, list(range(N_CORES)), trace=trace)
    stats_list = [r["stats"] for r in res.results]
    return stats_list, res


def kernel(predictions, Diss, target, diff, sigma):
    predictions = np.asarray(predictions)
    Diss = np.asarray(Diss)
    target = np.asarray(target)
    stats_list, _ = run_on_hw(predictions, Diss, target, trace=False)
    return assemble(stats_list, np.asarray(sigma), np.asarray(diff))


# revision 11
# speedup vs baseline: 1.1084x; 1.1084x over previous
"""Trainium2 Bass kernel for nn_CombineLoss (focal + dice + edge loss).

Sharding: data-parallel over the batch dim B=8 -> one batch image per
NeuronCore.  Each core computes 9 partial sums per 128-row tile (8 tiles);
the host combines them in float64.

Math notes (per head; heads = 2 softmaxed prediction heads + 1 raw-prob head):
  - softmax over C=2 channels:  p1 = sigmoid(x1-x0), p0 = sigmoid(x0-x1),
    so log p_t = log(sigmoid((2t-1)*(x1-x0))).
  - log fusion: focal and edge only need sum_h log p_t^(h) (sigma weights
    shared across heads), so ln(pt0*pt1*ptd) replaces three Ln ops.
  - 7x7 "circular" box filter (29 taps): decomposed by row offset dy:
      29*ave(p,x) = [H5(p,x) + t(p,x-3) + t(p,x+3)]   (dy=0 row, 7 taps)
                  + H5(p+/-1,x) + H5(p+/-2,x)         (|dy|=1,2 rows)
                  + t(p+/-3,x)                        (|dy|=3 rows)
    where H5 = 5-wide horizontal window sum (computed with 4 shifted bf16
    adds, exact small integers).  The vertical (partition) combination runs
    on the TensorEngine as banded matmuls; the -29*t center term is folded
    into the A33 center band so PSUM ends up holding 29*(ave-t) and
    |PSUM| = 29*at.
  - scalar-engine work is chunked (all sigmoids of a 4-tile chunk, then all
    Lns) because no activation table holds both Sigmoid and Ln.
  - per-tile sums ride on activation / tensor_scalar / scalar_tensor_tensor
    accum_out; final reduction happens on host in float64.
"""

import numpy as np

# ---------------------------------------------------------------------------
# problem constants (hardcoded per contest rules)
N_HEADS_PRED = 2   # predictions list dim
B = 8              # batch -> one per core
C = 2
H = 1024
W = 1024
P = 128            # partitions
NT = H // P        # 8 row tiles per image
TBW = 1032         # padded width of the bf16 target slots (3 left, 5 right)
SLOTS = 9          # stat slots per tile
NSLOT = NT * SLOTS
EPS = 1e-10
N_CORES = 8
CHUNK = 4          # tiles per scalar-table chunk (sigmoids ... lns)

# stat slot layout (per tile, base = 9*i):
#  +0  sum(t)
#  +1  pred0 sum(pt)     +2  pred1 sum(pt)
#  +3  sum(lp_all)       (ln(pt0*pt1*ptd), all 3 heads)
#  +4  pred0 sum(pt*t)   +5  pred1 sum(pt*t)
#  +6  sum(lp_all*at29)
#  +7  diss  sum(p1)     +8  diss  sum(p1*t)

_CACHE = {}


def _ensure_path():
    try:
        import concourse  # noqa: F401
    except ImportError:
        import sys
        for p in ("/opt/trn_rl_repo", "/root/.axon_site/_ro/trn_rl_repo"):
            if p not in sys.path:
                sys.path.insert(0, p)


# band indices
A25C, A25P, A25N, A33C, A33P, A33N, I128 = range(7)


def _make_bands():
    """7 [128,128] banded matrices, bf16 (all entries small ints -> exact).

    Built as stat[m, p]: contraction index m (source row), output partition p.
    matmul computes out[p, j] = sum_m stat[m, p] * moving[m, j].
    """
    import ml_dtypes
    idx = np.arange(P)
    d = idx[:, None] - idx[None, :]  # d[m, p] = m - p
    a25c = (np.abs(d) <= 2).astype(np.float32)
    a25p = ((d >= 126) & (d <= 127)).astype(np.float32)
    a25n = ((d <= -126) & (d >= -127)).astype(np.float32)
    a33c = (np.abs(d) == 3).astype(np.float32) - 29.0 * (d == 0)
    a33p = (d == 125).astype(np.float32)
    a33n = (d == -125).astype(np.float32)
    i128 = (d == 0).astype(np.float32)
    bands = np.stack([a25c, a25p, a25n, a33c, a33p, a33n, i128])
    return bands.astype(ml_dtypes.bfloat16)


def _build_nc():
    _ensure_path()
    import concourse.mybir as mybir
    from concourse import bacc
    from concourse.tile import TileContext

    f32 = mybir.dt.float32
    bf16 = mybir.dt.bfloat16
    i32 = mybir.dt.int32
    Alu = mybir.AluOpType
    Act = mybir.ActivationFunctionType

    nc = bacc.Bacc()
    preds = nc.dram_tensor("preds", [N_HEADS_PRED, C, H, W], f32,
                           kind="ExternalInput")
    diss = nc.dram_tensor("diss", [C, H, W], f32, kind="ExternalInput")
    target = nc.dram_tensor("target", [H, W], i32, kind="ExternalInput")
    bands = nc.dram_tensor("bands", [7, P, P], bf16, kind="ExternalInput")
    stats_out = nc.dram_tensor("stats", [P, NSLOT], f32, kind="ExternalOutput")

    with TileContext(nc) as tc:
        with (
            tc.tile_pool(name="const", bufs=1) as constp,
            tc.tile_pool(name="res", bufs=1) as resp,
            tc.tile_pool(name="io", bufs=9) as iop,
            tc.tile_pool(name="t32pool", bufs=3) as t32p,
            tc.tile_pool(name="mid", bufs=6) as midp,
            tc.tile_pool(name="keep", bufs=5) as keepp,
            tc.tile_pool(name="ps", bufs=2, space="PSUM") as psump,
        ):
            stats_sb = constp.tile([P, NSLOT], f32)
            eps_t = constp.tile([P, 1], f32)
            nc.gpsimd.memset(eps_t[:], EPS)
            tb_all = resp.tile([P, NT, TBW], bf16)
            h5_all = resp.tile([P, NT, W], bf16)
            nc.gpsimd.memset(tb_all[:], 0.0)
            ones_sb = constp.tile([P, W], bf16)
            nc.gpsimd.memset(ones_sb[:], 1.0)

            # per-tile state captured by phaseA, consumed by later stages
            state = {}

            def phase_a(i):
                """DMA loads + target-derived planes (tb, sgn, H5)."""
                t32 = t32p.tile([P, W], i32, tag="t32", name=f"t32_{i}")
                nc.sync.dma_start(out=t32, in_=target[i * P:(i + 1) * P, :])
                xt0 = iop.tile([P, C, W], f32, tag="xin", name=f"xt0_{i}")
                nc.sync.dma_start(
                    out=xt0, in_=preds[0, :, i * P:(i + 1) * P, :].rearrange(
                        "c h w -> h c w"))
                xt1 = iop.tile([P, C, W], f32, tag="xin", name=f"xt1_{i}")
                nc.sync.dma_start(
                    out=xt1, in_=preds[1, :, i * P:(i + 1) * P, :].rearrange(
                        "c h w -> h c w"))
                dt = iop.tile([P, C, W], f32, tag="xin", name=f"dt_{i}")
                nc.sync.dma_start(
                    out=dt, in_=diss[:, i * P:(i + 1) * P, :].rearrange(
                        "c h w -> h c w"))

                base = i * SLOTS
                tb_i = tb_all[:, i, 3:3 + W]
                # i32 -> bf16 {0,1} plane, accumulate sum(t) (Copy: no table)
                nc.scalar.activation(
                    tb_i, t32, Act.Copy,
                    accum_out=stats_sb[:, base + 0:base + 1])
                sgn = midp.tile([P, W], bf16, tag="sgn", bufs=3,
                                name=f"sgn_{i}")
                nc.gpsimd.tensor_scalar(sgn, t32, 2.0, -1.0,
                                        Alu.mult, Alu.add)
                # H5 = 5-wide horizontal sum of t, exact ints in bf16.
                # 3 ops via pair-reuse: P2(x)=t(x)+t(x+1) on a padded range,
                # P4(x)=P2(x-2)+P2(x), H5=P4+t(x+2).
                tp = tb_all[:, i, :]
                p2 = midp.tile([P, W + 4], bf16, tag="p2", bufs=2,
                               name=f"p2_{i}")  # col j <-> x = j-2
                nc.vector.tensor_tensor(
                    p2, tp[:, 1:1 + W + 4], tp[:, 2:2 + W + 4], Alu.add)
                p4 = midp.tile([P, W], bf16, tag="scr", name=f"p4_{i}")
                nc.vector.tensor_tensor(p4, p2[:, 0:W], p2[:, 2:2 + W],
                                        Alu.add)
                nc.vector.tensor_tensor(h5_all[:, i, :], p4,
                                        tp[:, 5:5 + W], Alu.add)
                state[i] = (t32, xt0, xt1, dt, sgn)

            def conv_elem(i):
                """Conv on PE + elementwise pipeline up to prod/at29."""
                base = i * SLOTS
                t32, xt0, xt1, dt, sgn = state.pop(i)
                tb_i = tb_all[:, i, 3:3 + W]

                # PE: psum = 29*(ave - t), exact integers
                vps = psump.tile([P, W], f32, tag="vps", name=f"vps_{i}")
                for h in range(2):
                    w0 = h * 512
                    mms = [(A25C, h5_all[:, i, w0:w0 + 512])]
                    if i > 0:
                        mms.append((A25P, h5_all[:, i - 1, w0:w0 + 512]))
                    if i < NT - 1:
                        mms.append((A25N, h5_all[:, i + 1, w0:w0 + 512]))
                    mms.append((A33C, tb_all[:, i, 3 + w0:3 + w0 + 512]))
                    if i > 0:
                        mms.append((A33P, tb_all[:, i - 1, 3 + w0:3 + w0 + 512]))
                    if i < NT - 1:
                        mms.append((A33N, tb_all[:, i + 1, 3 + w0:3 + w0 + 512]))
                    # dy=0 row: t(x-3) + t(x+3) via identity band
                    mms.append((I128, tb_all[:, i, w0:w0 + 512]))
                    mms.append((I128, tb_all[:, i, 6 + w0:6 + w0 + 512]))
                    for j, (bk, mv) in enumerate(mms):
                        nc.tensor.matmul(
                            vps[:, w0:w0 + 512], bands_sb[:, bk, :], mv,
                            start=(j == 0), stop=(j == len(mms) - 1))

                at29 = keepp.tile([P, W], bf16, tag="at29", name=f"at29_{i}")
                nc.scalar.activation(at29, vps, Act.Abs)  # = 29*|t-ave|

                # s = (x1-x0)*sgn per pred head; d on Pool, s on DVE
                d0 = midp.tile([P, W], bf16, tag="scr", name=f"d0_{i}")
                nc.gpsimd.tensor_tensor(
                    d0, xt0[:, 1, :], xt0[:, 0, :], Alu.subtract)
                s0 = midp.tile([P, W], bf16, tag="s", bufs=3, name=f"s0_{i}")
                nc.vector.tensor_tensor(s0, d0, sgn, Alu.mult)
                d1 = midp.tile([P, W], bf16, tag="scr", name=f"d1_{i}")
                nc.gpsimd.tensor_tensor(
                    d1, xt1[:, 1, :], xt1[:, 0, :], Alu.subtract)
                s1 = midp.tile([P, W], bf16, tag="s", bufs=3, name=f"s1_{i}")
                nc.vector.tensor_tensor(s1, d1, sgn, Alu.mult)

                pt0 = midp.tile([P, W], bf16, tag="pt", bufs=4,
                                name=f"pt0_{i}")
                nc.scalar.activation(
                    pt0, s0, Act.Sigmoid,
                    accum_out=stats_sb[:, base + 1:base + 2])
                pt1 = midp.tile([P, W], bf16, tag="pt", bufs=4,
                                name=f"pt1_{i}")
                nc.scalar.activation(
                    pt1, s1, Act.Sigmoid,
                    accum_out=stats_sb[:, base + 2:base + 3])

                qt0 = midp.tile([P, W], bf16, tag="scr", name=f"qt0_{i}")
                nc.vector.affine_mul_reduce(
                    out=qt0, accum_out=stats_sb[:, base + 4:base + 5],
                    in0=pt0, in1=tb_i, scale=1.0, bias=0.0)
                qt1 = midp.tile([P, W], bf16, tag="scr", name=f"qt1_{i}")
                nc.vector.affine_mul_reduce(
                    out=qt1, accum_out=stats_sb[:, base + 5:base + 6],
                    in0=pt1, in1=tb_i, scale=1.0, bias=0.0)

                # diss: ptd = t ? dt1 : dt0 (exact select); copy on Scalar
                ptd = midp.tile([P, W], f32, tag="ptd", bufs=3,
                                name=f"ptd_{i}")
                nc.scalar.activation(ptd, dt[:, 0, :], Act.Copy)
                nc.vector.copy_predicated(ptd, t32, dt[:, 1, :])
                pp = midp.tile([P, W], bf16, tag="scr", name=f"pp_{i}")
                nc.vector.tensor_tensor(pp, pt0, pt1, Alu.mult)
                prod = keepp.tile([P, W], bf16, tag="prod", name=f"prod_{i}")
                nc.vector.tensor_tensor(prod, pp, ptd, Alu.mult)

                # diss dice sums on DVE (accum-reduce is DVE-only)
                scr7 = midp.tile([P, W], bf16, tag="scr", name=f"scr7_{i}")
                nc.vector.affine_mul_reduce(
                    out=scr7, accum_out=stats_sb[:, base + 7:base + 8],
                    in0=dt[:, 1, :], in1=ones_sb, scale=1.0, bias=0.0)
                scr8 = midp.tile([P, W], bf16, tag="scr", name=f"scr8_{i}")
                nc.vector.affine_mul_reduce(
                    out=scr8, accum_out=stats_sb[:, base + 8:base + 9],
                    in0=dt[:, 1, :], in1=tb_i, scale=1.0, bias=0.0)
                state[("pr", i)] = (prod, at29)

            def phase_ln(i):
                """Deferred Ln + edge product-reduce (chunked for tables)."""
                base = i * SLOTS
                prod, at29 = state.pop(("pr", i))
                lnp = midp.tile([P, W], bf16, tag="lnp", bufs=2,
                                name=f"lnp_{i}")
                nc.scalar.activation(
                    lnp, prod, Act.Ln, bias=eps_t[:, 0:1],
                    accum_out=stats_sb[:, base + 3:base + 4])
                escr = midp.tile([P, W], bf16, tag="scr", name=f"escr_{i}")
                nc.vector.affine_mul_reduce(
                    out=escr, accum_out=stats_sb[:, base + 6:base + 7],
                    in0=lnp, in1=at29, scale=1.0, bias=0.0)

            # ---- issue the pipeline ------------------------------------
            phase_a(0)
            bands_sb = constp.tile([P, 7, P], bf16)
            for k in range(7):
                nc.sync.dma_start(out=bands_sb[:, k, :], in_=bands[k])
            for i in range(NT):
                if i + 1 < NT:
                    phase_a(i + 1)
                conv_elem(i)
                if i % CHUNK == CHUNK - 1:
                    for j in range(i - CHUNK + 1, i + 1):
                        phase_ln(j)

            nc.sync.dma_start(out=stats_out[:], in_=stats_sb[:])

    nc.finalize()
    return nc


def get_program():
    if "nc" not in _CACHE:
        _CACHE["nc"] = _build_nc()
    return _CACHE["nc"]


def make_in_maps(predictions, Diss, target):
    bands = _make_bands()
    in_maps = []
    for c in range(N_CORES):
        in_maps.append({
            "preds": np.ascontiguousarray(predictions[:, c], dtype=np.float32),
            "diss": np.ascontiguousarray(Diss[0, c], dtype=np.float32),
            "target": np.ascontiguousarray(target[c], dtype=np.int32),
            "bands": bands,
        })
    return in_maps


def assemble(stats_list, sigma, diff):
    """Combine per-core [128, 72] stats into the scalar loss (float64)."""
    HW = float(H * W)
    focal_all = 0.0
    edge_all = 0.0
    dice_sum = 0.0  # sum over cores and heads of 2I/(U+eps)
    for st in stats_list:
        g = st.astype(np.float64).sum(axis=0).reshape(NT, SLOTS).sum(axis=0)
        s_t = g[0]
        n0 = HW - s_t
        focal_all += -g[3]
        edge_all += -g[6] / 29.0
        for hh in range(2):
            spt, i_sum = g[1 + hh], g[4 + hh]
            sp1 = n0 + 2.0 * i_sum - spt
            dice_sum += 2.0 * i_sum / (sp1 + s_t + EPS)
        sp1d, i_d = g[7], g[8]
        dice_sum += 2.0 * i_d / (sp1d + s_t + EPS)

    sig2 = np.asarray(sigma, np.float64) ** 2
    denom = float(N_CORES) * HW
    loss = (focal_all / denom / sig2[0]
            + (3.0 - dice_sum / float(N_CORES)) / sig2[1]
            + edge_all / denom / sig2[2])
    loss += float(diff)
    loss += float(np.sum(np.log(sig2))) / 2.0
    return np.float32(loss)


def run_on_hw(predictions, Diss, target, trace=False):
    _ensure_path()
    from concourse.bass_utils import run_bass_kernel_spmd
    nc = get_program()
    in_maps = make_in_maps(predictions, Diss, target)
    res = run_bass_kernel_spmd(nc, in_maps, list(range(N_CORES)), trace=trace)
    stats_list = [r["stats"] for r in res.results]
    return stats_list, res


def kernel(predictions, Diss, target, diff, sigma):
    predictions = np.asarray(predictions)
    Diss = np.asarray(Diss)
    target = np.asarray(target)
    stats_list, _ = run_on_hw(predictions, Diss, target, trace=False)
    return assemble(stats_list, np.asarray(sigma), np.asarray(diff))


# revision 14
# speedup vs baseline: 1.1534x; 1.0407x over previous
"""Trainium2 Bass kernel for nn_CombineLoss (focal + dice + edge loss).

Sharding: data-parallel over the batch dim B=8 -> one batch image per
NeuronCore.  Each core computes 9 partial sums per 256-row pair-tile
(4 pairs); the host combines them in float64.

Math notes (per head; heads = 2 softmaxed prediction heads + 1 raw-prob head):
  - softmax over C=2 channels:  p1 = sigmoid(x1-x0), p0 = sigmoid(x0-x1),
    so log p_t = log(sigmoid((2t-1)*(x1-x0))).
  - log fusion: focal and edge only need sum_h log p_t^(h) (sigma weights
    shared across heads), so ln(pt0*pt1*ptd) replaces three Ln ops.
  - 7x7 "circular" box filter (29 taps): decomposed by row offset dy:
      29*ave(p,x) = [H5(p,x) + t(p,x-3) + t(p,x+3)]   (dy=0 row, 7 taps)
                  + H5(p+/-1,x) + H5(p+/-2,x)         (|dy|=1,2 rows)
                  + t(p+/-3,x)                        (|dy|=3 rows)
    H5 = 5-wide horizontal window sum (3 shifted bf16 adds via pair reuse,
    exact small integers).  The vertical (partition) combination runs on
    the TensorEngine as banded matmuls; the -29*t center term is folded
    into the A33 center band so PSUM holds 29*(ave-t) and |PSUM| = 29*at.
  - engine split: DVE takes adds/muls + all accum-reduces (affine_mul_reduce)
    + the predicated select; Pool (gpsimd) takes sgn and one subtract; the
    Scalar engine takes sigmoid/ln/abs/copies (Copy+Abs live in every
    activation table, so only sigmoid<->ln chunk swaps reload tables).
  - ops are issued on 2-tile (256-row) views to amortize the ~0.6us
    fixed per-instruction cost of the vector engines.
"""

import numpy as np

# ---------------------------------------------------------------------------
# problem constants (hardcoded per contest rules)
N_HEADS_PRED = 2   # predictions list dim
B = 8              # batch -> one per core
C = 2
H = 1024
W = 1024
P = 128            # partitions
NT = H // P        # 8 row tiles per image
NPAIR = NT // 2    # 4 pair-tiles
TBW = 1032         # padded width of the bf16 target slots (3 left, 5 right)
SLOTS = 9          # stat slots per pair
NSLOT = NPAIR * SLOTS
EPS = 1e-10
N_CORES = 8

# stat slot layout (per pair, base = 9*j):
#  +0  sum(t)
#  +1  pred0 sum(pt)     +2  pred1 sum(pt)
#  +3  sum(lp_all)       (ln(pt0*pt1*ptd), all 3 heads)
#  +4  pred0 sum(pt*t)   +5  pred1 sum(pt*t)
#  +6  sum(lp_all*at29)
#  +7  diss  sum(p1)     +8  diss  sum(p1*t)

_CACHE = {}


def _ensure_path():
    try:
        import concourse  # noqa: F401
    except ImportError:
        import sys
        for p in ("/opt/trn_rl_repo", "/root/.axon_site/_ro/trn_rl_repo"):
            if p not in sys.path:
                sys.path.insert(0, p)


# band indices
A25C, A25P, A25N, A33C, A33P, A33N, I128 = range(7)


def _make_bands():
    """7 [128,128] banded matrices, bf16 (all entries small ints -> exact).

    Built as stat[m, p]: contraction index m (source row), output partition p.
    matmul computes out[p, j] = sum_m stat[m, p] * moving[m, j].
    """
    import ml_dtypes
    idx = np.arange(P)
    d = idx[:, None] - idx[None, :]  # d[m, p] = m - p
    a25c = (np.abs(d) <= 2).astype(np.float32)
    a25p = ((d >= 126) & (d <= 127)).astype(np.float32)
    a25n = ((d <= -126) & (d >= -127)).astype(np.float32)
    a33c = (np.abs(d) == 3).astype(np.float32) - 29.0 * (d == 0)
    a33p = (d == 125).astype(np.float32)
    a33n = (d == -125).astype(np.float32)
    i128 = (d == 0).astype(np.float32)
    bands = np.stack([a25c, a25p, a25n, a33c, a33p, a33n, i128])
    return bands.astype(ml_dtypes.bfloat16)


def _build_nc():
    _ensure_path()
    import concourse.mybir as mybir
    from concourse import bacc
    from concourse.tile import TileContext

    f32 = mybir.dt.float32
    bf16 = mybir.dt.bfloat16
    i32 = mybir.dt.int32
    Alu = mybir.AluOpType
    Act = mybir.ActivationFunctionType

    nc = bacc.Bacc()
    preds = nc.dram_tensor("preds", [N_HEADS_PRED, C, H, W], f32,
                           kind="ExternalInput")
    diss = nc.dram_tensor("diss", [C, H, W], f32, kind="ExternalInput")
    target = nc.dram_tensor("target", [H, W], i32, kind="ExternalInput")
    bands = nc.dram_tensor("bands", [7, P, P], bf16, kind="ExternalInput")
    stats_out = nc.dram_tensor("stats", [P, NSLOT], f32, kind="ExternalOutput")

    with TileContext(nc) as tc:
        with (
            tc.tile_pool(name="const", bufs=1) as constp,
            tc.tile_pool(name="res", bufs=1) as resp,
            tc.tile_pool(name="io", bufs=4) as iop,
            tc.tile_pool(name="t32pool", bufs=3) as t32p,
            tc.tile_pool(name="mid", bufs=3) as midp,
            tc.tile_pool(name="keep", bufs=2) as keepp,
            tc.tile_pool(name="ps", bufs=2, space="PSUM") as psump,
        ):
            stats_sb = constp.tile([P, NSLOT], f32)
            eps_t = constp.tile([P, 1], f32)
            nc.gpsimd.memset(eps_t[:], EPS)
            ones2 = constp.tile([P, 2, W], bf16)
            nc.gpsimd.memset(ones2[:], 1.0)
            tb_all = resp.tile([P, NT, TBW], bf16)
            nc.gpsimd.memset(tb_all[:], 0.0)

            state = {}
            h5p = {}

            def h5_tile(i):
                """[P, W] view of H5 for row-tile i from the pair pool."""
                return h5p[i // 2][:, i % 2, :]

            def phase_a(j):
                """Pair j (tiles 2j, 2j+1): DMA + target planes tb/sgn/H5."""
                r0 = j * 2 * P  # first image row of the pair
                t32 = t32p.tile([P, 2, W], i32, tag="t32", name=f"t32_{j}")
                nc.sync.dma_start(
                    out=t32, in_=target[r0:r0 + 2 * P, :].rearrange(
                        "(t h) w -> h t w", t=2))
                xt0 = iop.tile([P, 2, C, W], f32, tag="xin", name=f"xt0_{j}")
                xt1 = iop.tile([P, 2, C, W], f32, tag="xin", name=f"xt1_{j}")
                dt = iop.tile([P, 2, C, W], f32, tag="xin", name=f"dt_{j}")
                for k in range(2):
                    rk = r0 + k * P
                    nc.sync.dma_start(
                        out=xt0[:, k, :, :],
                        in_=preds[0, :, rk:rk + P, :].rearrange(
                            "c h w -> h c w"))
                    nc.sync.dma_start(
                        out=xt1[:, k, :, :],
                        in_=preds[1, :, rk:rk + P, :].rearrange(
                            "c h w -> h c w"))
                    nc.sync.dma_start(
                        out=dt[:, k, :, :],
                        in_=diss[:, rk:rk + P, :].rearrange(
                            "c h w -> h c w"))

                base = j * SLOTS
                i0 = 2 * j
                tb_j = tb_all[:, i0:i0 + 2, 3:3 + W]
                # i32 -> bf16 {0,1} planes, accumulate sum(t) (Copy: no table)
                nc.scalar.activation(
                    tb_j, t32, Act.Copy,
                    accum_out=stats_sb[:, base + 0:base + 1])
                sgn = midp.tile([P, 2, W], bf16, tag="sgn", bufs=2,
                                name=f"sgn_{j}")
                nc.gpsimd.tensor_scalar(sgn, t32, 2.0, -1.0,
                                        Alu.mult, Alu.add)
                # H5 = 5-wide horizontal sum of t, exact ints in bf16.
                # P2(x)=t(x)+t(x+1) on a padded range (col k <-> x = k-2),
                # P4(x)=P2(x-2)+P2(x), H5=P4+t(x+2).
                tp = tb_all[:, i0:i0 + 2, :]
                p2 = midp.tile([P, 2, W + 4], bf16, tag="p2", bufs=2,
                               name=f"p2_{j}")
                nc.vector.tensor_tensor(
                    p2, tp[:, :, 1:1 + W + 4], tp[:, :, 2:2 + W + 4], Alu.add)
                p4 = midp.tile([P, 2, W], bf16, tag="scr", name=f"p4_{j}")
                nc.vector.tensor_tensor(p4, p2[:, :, 0:W], p2[:, :, 2:2 + W],
                                        Alu.add)
                h5t = midp.tile([P, 2, W], bf16, tag="h5", bufs=3,
                                name=f"h5_{j}")
                nc.vector.tensor_tensor(h5t, p4, tp[:, :, 5:5 + W], Alu.add)
                h5p[j] = h5t
                state[j] = (t32, xt0, xt1, dt, sgn)

            def conv_elem(j):
                """Conv on PE + elementwise pipeline for pair j."""
                base = j * SLOTS
                i0 = 2 * j
                t32, xt0, xt1, dt, sgn = state.pop(j)
                tb_j = tb_all[:, i0:i0 + 2, 3:3 + W]

                # PE: psum = 29*(ave - t), exact integers; per 512-col segment
                vps = psump.tile([P, 2, W], f32, tag="vps", name=f"vps_{j}")
                for k in range(2):
                    i = i0 + k
                    for h in range(2):
                        w0 = h * 512
                        mms = [(A25C, h5_tile(i)[:, w0:w0 + 512])]
                        if i > 0:
                            mms.append((A25P, h5_tile(i - 1)[:, w0:w0 + 512]))
                        if i < NT - 1:
                            mms.append((A25N, h5_tile(i + 1)[:, w0:w0 + 512]))
                        mms.append((A33C, tb_all[:, i, 3 + w0:3 + w0 + 512]))
                        if i > 0:
                            mms.append(
                                (A33P, tb_all[:, i - 1, 3 + w0:3 + w0 + 512]))
                        if i < NT - 1:
                            mms.append(
                                (A33N, tb_all[:, i + 1, 3 + w0:3 + w0 + 512]))
                        # dy=0 row: t(x-3) + t(x+3) via identity band
                        mms.append((I128, tb_all[:, i, w0:w0 + 512]))
                        mms.append((I128, tb_all[:, i, 6 + w0:6 + w0 + 512]))
                        for m, (bk, mv) in enumerate(mms):
                            nc.tensor.matmul(
                                vps[:, k, w0:w0 + 512], bands_sb[:, bk, :],
                                mv, start=(m == 0), stop=(m == len(mms) - 1))

                at29 = keepp.tile([P, 2, W], bf16, tag="at29",
                                  name=f"at29_{j}")
                nc.scalar.activation(at29, vps, Act.Abs)  # = 29*|t-ave|

                # s = (x1-x0)*sgn per pred head; d0 on Pool, rest on DVE
                d0 = midp.tile([P, 2, W], bf16, tag="scr", name=f"d0_{j}")
                nc.gpsimd.tensor_tensor(
                    d0, xt0[:, :, 1, :], xt0[:, :, 0, :], Alu.subtract)
                s0 = midp.tile([P, 2, W], bf16, tag="s", bufs=2,
                               name=f"s0_{j}")
                nc.vector.tensor_tensor(s0, d0, sgn, Alu.mult)
                d1 = midp.tile([P, 2, W], bf16, tag="scr", name=f"d1_{j}")
                nc.vector.tensor_tensor(
                    d1, xt1[:, :, 1, :], xt1[:, :, 0, :], Alu.subtract)
                s1 = midp.tile([P, 2, W], bf16, tag="s", bufs=2,
                               name=f"s1_{j}")
                nc.vector.tensor_tensor(s1, d1, sgn, Alu.mult)

                pt0 = midp.tile([P, 2, W], bf16, tag="pt", bufs=2,
                                name=f"pt0_{j}")
                nc.scalar.activation(
                    pt0, s0, Act.Sigmoid,
                    accum_out=stats_sb[:, base + 1:base + 2])
                pt1 = midp.tile([P, 2, W], bf16, tag="pt", bufs=2,
                                name=f"pt1_{j}")
                nc.scalar.activation(
                    pt1, s1, Act.Sigmoid,
                    accum_out=stats_sb[:, base + 2:base + 3])

                qt0 = midp.tile([P, 2, W], bf16, tag="scr", name=f"qt0_{j}")
                nc.vector.affine_mul_reduce(
                    out=qt0, accum_out=stats_sb[:, base + 4:base + 5],
                    in0=pt0, in1=tb_j, scale=1.0, bias=0.0)
                qt1 = midp.tile([P, 2, W], bf16, tag="scr", name=f"qt1_{j}")
                nc.vector.affine_mul_reduce(
                    out=qt1, accum_out=stats_sb[:, base + 5:base + 6],
                    in0=pt1, in1=tb_j, scale=1.0, bias=0.0)

                # diss: ptd = t ? dt1 : dt0 (exact select); copy on Scalar
                ptd = midp.tile([P, 2, W], f32, tag="ptd", bufs=2,
                                name=f"ptd_{j}")
                nc.scalar.activation(ptd, dt[:, :, 0, :], Act.Copy)
                nc.vector.copy_predicated(ptd, t32, dt[:, :, 1, :])
                pp = midp.tile([P, 2, W], bf16, tag="scr", name=f"pp_{j}")
                nc.vector.tensor_tensor(pp, pt0, pt1, Alu.mult)
                prod = keepp.tile([P, 2, W], bf16, tag="prod",
                                  name=f"prod_{j}")
                nc.vector.tensor_tensor(prod, pp, ptd, Alu.mult)

                # diss dice sums (accum-reduce is DVE-only)
                scr7 = midp.tile([P, 2, W], bf16, tag="scr", name=f"scr7_{j}")
                nc.vector.affine_mul_reduce(
                    out=scr7, accum_out=stats_sb[:, base + 7:base + 8],
                    in0=dt[:, :, 1, :], in1=ones2, scale=1.0, bias=0.0)
                scr8 = midp.tile([P, 2, W], bf16, tag="scr", name=f"scr8_{j}")
                nc.vector.affine_mul_reduce(
                    out=scr8, accum_out=stats_sb[:, base + 8:base + 9],
                    in0=dt[:, :, 1, :], in1=tb_j, scale=1.0, bias=0.0)
                state[("pr", j)] = (prod, at29)

            def phase_ln(j):
                """Deferred Ln + edge product-reduce (chunked for tables)."""
                base = j * SLOTS
                prod, at29 = state.pop(("pr", j))
                lnp = midp.tile([P, 2, W], bf16, tag="lnp", bufs=2,
                                name=f"lnp_{j}")
                nc.scalar.activation(
                    lnp, prod, Act.Ln, bias=eps_t[:, 0:1],
                    accum_out=stats_sb[:, base + 3:base + 4])
                escr = midp.tile([P, 2, W], bf16, tag="scr", name=f"escr_{j}")
                nc.vector.affine_mul_reduce(
                    out=escr, accum_out=stats_sb[:, base + 6:base + 7],
                    in0=lnp, in1=at29, scale=1.0, bias=0.0)

            # ---- issue the pipeline ------------------------------------
            phase_a(0)
            bands_sb = constp.tile([P, 7, P], bf16)
            for k in range(7):
                nc.sync.dma_start(out=bands_sb[:, k, :], in_=bands[k])
            for j in range(NPAIR):
                if j + 1 < NPAIR:
                    phase_a(j + 1)
                conv_elem(j)
                if j % 2 == 1:
                    phase_ln(j - 1)
                    phase_ln(j)

            nc.sync.dma_start(out=stats_out[:], in_=stats_sb[:])

    nc.finalize()
    return nc


def get_program():
    if "nc" not in _CACHE:
        _CACHE["nc"] = _build_nc()
    return _CACHE["nc"]


def make_in_maps(predictions, Diss, target):
    bands = _make_bands()
    in_maps = []
    for c in range(N_CORES):
        in_maps.append({
            "preds": np.ascontiguousarray(predictions[:, c], dtype=np.float32),
            "diss": np.ascontiguousarray(Diss[0, c], dtype=np.float32),
            "target": np.ascontiguousarray(target[c], dtype=np.int32),
            "bands": bands,
        })
    return in_maps


def assemble(stats_list, sigma, diff):
    """Combine per-core [128, 36] stats into the scalar loss (float64)."""
    HW = float(H * W)
    focal_all = 0.0
    edge_all = 0.0
    dice_sum = 0.0  # sum over cores and heads of 2I/(U+eps)
    for st in stats_list:
        g = st.astype(np.float64).sum(axis=0).reshape(NPAIR, SLOTS).sum(axis=0)
        s_t = g[0]
        n0 = HW - s_t
        focal_all += -g[3]
        edge_all += -g[6] / 29.0
        for hh in range(2):
            spt, i_sum = g[1 + hh], g[4 + hh]
            sp1 = n0 + 2.0 * i_sum - spt
            dice_sum += 2.0 * i_sum / (sp1 + s_t + EPS)
        sp1d, i_d = g[7], g[8]
        dice_sum += 2.0 * i_d / (sp1d + s_t + EPS)

    sig2 = np.asarray(sigma, np.float64) ** 2
    denom = float(N_CORES) * HW
    loss = (focal_all / denom / sig2[0]
            + (3.0 - dice_sum / float(N_CORES)) / sig2[1]
            + edge_all / denom / sig2[2])
    loss += float(diff)
    loss += float(np.sum(np.log(sig2))) / 2.0
    return np.float32(loss)


def run_on_hw(predictions, Diss, target, trace=False):
    _ensure_path()
    from concourse.bass_utils import run_bass_kernel_spmd
    nc = get_program()
    in_maps = make_in_maps(predictions, Diss, target)
    res = run_bass_kernel_spmd(nc, in_maps, list(range(N_CORES)), trace=trace)
    stats_list = [r["stats"] for r in res.results]
    return stats_list, res


def kernel(predictions, Diss, target, diff, sigma):
    predictions = np.asarray(predictions)
    Diss = np.asarray(Diss)
    target = np.asarray(target)
    stats_list, _ = run_on_hw(predictions, Diss, target, trace=False)
    return assemble(stats_list, np.asarray(sigma), np.asarray(diff))
